# revision 63
# baseline (speedup 1.0000x reference)
"""Per-pixel predicted 5x5 conv (KPN-style) on 8 trn2 cores.

Sharding: data-parallel over (batch x H-half) = 8 shards of 128 output rows.

Design (v3):
  - feat loaded ONCE per core as 2 w-parity copies (4B-aligned stride-1 runs
    keep DVE tensor_tensor in 2x_1P bf16 mode for every tap's w-shift).
  - kern rows pre-shifted HOST-side per tap: KS[p, t] = kern[h0 + p - di(t), t],
    so prod_t[p] = featA[p] * KS[t, p] pairs the right kernel row with the
    right (h-shifted) feat row.
  - h-shift happens inside the PE accumulate via 5 shifted-identity
    stationaries S_di[p, po] = (p == po + di), masked to valid po < 128 - di.
    Stationary swaps cost nothing extra; PE streams each tap product once.
  - bottom-halo rows (po >= 128 - di) + bias are handled by ONE extra stacked
    matmul per psum bank: 50 (tap, halo-row) products + 1 bias row with a
    [51, 128] scatter stationary; the products are computed on DVE from
    host-prepped shifted operands.
  - 5 of the 25 tap products run on GPSIMD (Pool) to unload DVE.
  - DVE tap products are fused in pairs sharing di (same PE stationary) via
    custom overlapping access patterns -> half the DVE instruction count.
  - PSUM is split into a 3-bank and a 1-bank tile so the final quarter's
    small bank-3 group can close and drain independently (whole-tile psum
    dependency tracking otherwise serializes the tail).
"""

import sys

for p in ("/opt/pypackages", "/opt/trn_rl_repo"):
    if p not in sys.path:
        sys.path.insert(0, p)

import numpy as np
import ml_dtypes

import bass_rust
import concourse.bass as bass
import concourse.mybir as mybir
from concourse import bacc, tile
from concourse.bass_utils import run_bass_kernel_spmd

B, H, W, C, KK, K = 4, 256, 256, 32, 25, 5
HS = H // 2          # 128 output rows per core
CQ = 8               # channels per PSUM chunk (4 banks of 512 fp32)
NQ = C // CQ
NHALO = 51           # 50 (di,q<di,dj) fixup rows + 1 bias row
NH2 = 115            # halo rows split into two c-halves: [0:51] c0-3, [64:115] c4-7
POOL_TAPS = (2, 7, 12, 17, 22)   # dj=2 taps -> GPSIMD
BF16 = mybir.dt.bfloat16
F32 = mybir.dt.float32

# schedule items: ('pair', (ta, tb)) share di and parity; ('single', t);
# ('pool', t) is a product computed on GPSIMD.
_PAIRS0 = [(5 * di + 0, 5 * di + 4) for di in range(K)]   # dj 0,4 (parity 0)
_PAIRS1 = [(5 * di + 1, 5 * di + 3) for di in range(K)]   # dj 1,3 (parity 1)
SCHED_A = [('pair', _PAIRS0[0]), ('pool', 2),
           ('pair', _PAIRS0[1]), ('pool', 7),
           ('pair', _PAIRS0[2]), ('pool', 12),
           ('pair', _PAIRS0[3]), ('pool', 17),
           ('pair', _PAIRS0[4]),
           ('pair', _PAIRS1[0]), ('pool', 22),
           ('pair', _PAIRS1[1]), ('pair', _PAIRS1[2]),
           ('pair', _PAIRS1[3]), ('pair', _PAIRS1[4])]
# q0 variant with a 6th pool tap (14): q0's pools-late slots have the most
# deadline slack (PE p-state slow start + no competing prefetch)
SCHED_Z6 = [('single', 0), ('single', 4),
            ('pair', _PAIRS0[1]), ('single', 10),
            ('pool', 2),
            ('pair', _PAIRS0[3]), ('pair', _PAIRS0[4]),
            ('pool', 7),
            ('pair', _PAIRS1[0]), ('pair', _PAIRS1[1]),
            ('pool', 12),
            ('pair', _PAIRS1[2]), ('pair', _PAIRS1[3]),
            ('pool', 17),
            ('pool', 14),
            ('pair', _PAIRS1[4]),
            ('pool', 22)]
# pools-late for q1+ too: q0's extra pool op shifts the whole pool queue
SCHED_L = [('pair', _PAIRS0[0]), ('pair', _PAIRS0[1]), ('pair', _PAIRS0[2]),
           ('pool', 2),
           ('pair', _PAIRS0[3]), ('pair', _PAIRS0[4]),
           ('pool', 7),
           ('pair', _PAIRS1[0]), ('pair', _PAIRS1[1]),
           ('pool', 12),
           ('pair', _PAIRS1[2]), ('pair', _PAIRS1[3]),
           ('pool', 17),
           ('pair', _PAIRS1[4]),
           ('pool', 22)]
POOL_TAPS_Q = {0: (2, 7, 12, 17, 14, 22), 1: (2, 7, 12, 17, 22),
               2: (2, 7, 12, 17, 22), 3: (2, 7, 12, 17, 22)}
SCHED_Q = {0: SCHED_Z6, 1: SCHED_L, 2: SCHED_L, 3: SCHED_L}
# within each di-chunk, kernel rows are host-reordered [dj0, dj4, dj1, dj3,
# dj2] so both pair kinds read adjacent rows (stride W) and the pair-critical
# rows load first
DJ_ROW = {0: 0, 4: 1, 1: 2, 3: 3, 2: 4}
HALO_AFTER_Q = {0: 10, 1: 10, 2: 10, 3: 10}  # halo matmul position per quarter

_NC_CACHE = {}


def _pair_aps(fa_t, kb, ta, tb, c0=0, cn=CQ):
    """Overlapping-window APs computing both taps of a pair in one TT op,
    for channels [c0, c0+cn) of the quarter.

    in0[p, k, c, w] = fa_t[p, c0+c, off + k*step + w]  (k = 0/1 -> tap a/b)
    in1[p, k, c, w] = kb[p, k-th adjacent kernel row, w]  (c broadcast)
    """
    dja, djb = ta % K, tb % K
    par = dja % 2
    off = dja - par
    step = djb - dja                     # elements between the two w-shifts
    base = fa_t[:, c0:c0 + cn, off:off + W]
    a0 = base.ap
    in0 = bass.AP(tensor=base.tensor, offset=base.offset,
                  ap=bass_rust.VecI64Pair(
                      [list(a0[0]), [step, 2], list(a0[1]), list(a0[2])]))
    k0 = kb.ap
    in1 = bass.AP(tensor=kb.tensor, offset=kb.offset,
                  ap=bass_rust.VecI64Pair(
                      [list(k0[0]), [W, 2], [0, cn], list(k0[2])]))
    return in0, in1, par


def _build_nc():
    nc = bacc.Bacc(None, target_bir_lowering=False)
    fa0_d = nc.dram_tensor("fa0", [HS, C, W + 4], BF16, kind="ExternalInput")
    fa1_d = nc.dram_tensor("fa1", [HS, C, W + 4], BF16, kind="ExternalInput")
    ks_d = nc.dram_tensor("ks", [HS, KK, W], BF16, kind="ExternalInput")
    sst_d = nc.dram_tensor("sst", [HS, K * HS], BF16, kind="ExternalInput")
    shal_d = nc.dram_tensor("shal", [NH2, 2 * HS], BF16, kind="ExternalInput")
    fhs_d = nc.dram_tensor("fhs", [NH2, 16, W], BF16, kind="ExternalInput")
    kh_d = nc.dram_tensor("kh", [NH2, 1, W], BF16, kind="ExternalInput")
    out_d = nc.dram_tensor("out", [HS, C, W], BF16, kind="ExternalOutput")

    with tile.TileContext(nc) as tc:
        with tc.tile_pool(name="const", bufs=1) as cpool, \
             tc.tile_pool(name="dprod", bufs=6) as dpool, \
             tc.tile_pool(name="pprod", bufs=2) as ppool, \
             tc.tile_pool(name="hprod", bufs=2) as hpool, \
             tc.tile_pool(name="osb", bufs=4) as opool, \
             tc.tile_pool(name="psum", bufs=2, space="PSUM") as qpool:
            ks_a, ks_b = [], []
            for di in range(K):
                ka = cpool.tile([HS, 2, W], BF16, tag=f"ksa{di}",
                                name=f"ksa{di}")
                kb = cpool.tile([HS, 3, W], BF16, tag=f"ksb{di}",
                                name=f"ksb{di}")
                ks_a.append(ka)
                ks_b.append(kb)
            faq = {}
            for q in range(NQ):
                for par in range(2):
                    t_ = cpool.tile([HS, CQ, W + 4], BF16,
                                    tag=f"fa{par}q{q}", name=f"fa{par}q{q}")
                    faq[(par, q)] = t_
            fa_d = {0: fa0_d, 1: fa1_d}

            def load_faq(par, q):
                nc.sync.dma_start(
                    out=faq[(par, q)],
                    in_=fa_d[par][:, q * CQ:(q + 1) * CQ, :])

            def load_ks(di, half):
                if half == 0:
                    nc.sync.dma_start(out=ks_a[di],
                                      in_=ks_d[:, di * K:di * K + 2, :])
                else:
                    nc.sync.dma_start(out=ks_b[di],
                                      in_=ks_d[:, di * K + 2:di * K + K, :])

            # DMA order: quarter-0 critical operands first
            load_faq(0, 0)
            load_ks(0, 0)
            load_ks(0, 1)
            load_ks(1, 0)
            sst_t = cpool.tile([HS, K * HS], BF16, tag="sst")
            nc.sync.dma_start(out=sst_t, in_=sst_d[:, :])
            load_faq(1, 0)
            load_ks(1, 1)
            for di in range(2, K):
                load_ks(di, 0)
                load_ks(di, 1)
            load_faq(0, 1)
            load_faq(1, 1)
            fhs_t = cpool.tile([NH2, 16, W], BF16, tag="fhs")
            nc.sync.dma_start(out=fhs_t, in_=fhs_d[:, :, :])
            kh_t = cpool.tile([NH2, 1, W], BF16, tag="kh")
            nc.sync.dma_start(out=kh_t, in_=kh_d[:, :, :])
            shal_t = cpool.tile([NH2, 2 * HS], BF16, tag="shal")
            nc.sync.dma_start(out=shal_t, in_=shal_d[:, :])
            for q in range(2, NQ):
                load_faq(0, q)
                load_faq(1, q)

            def s_ap(di):
                return sst_t[:, di * HS:(di + 1) * HS]

            def ks_ap(t):
                di, dj = t // K, t % K
                row = DJ_ROW[dj]
                if row < 2:
                    return ks_a[di][:, row:row + 1, :]
                return ks_b[di][:, row - 2:row - 1, :]

            # pool products, prefetched one quarter ahead
            pool_prods = {}

            def issue_pool_quarter(qp):
                for t in POOL_TAPS_Q[qp]:
                    par = (t % K) % 2
                    off = t % K - par
                    pr = ppool.tile([HS, CQ, W], BF16, tag=f"pp{t}",
                                    name=f"pp{t}_{qp}")
                    nc.gpsimd.tensor_tensor(
                        pr,
                        faq[(par, qp)][:, :, off:off + W],
                        ks_ap(t).broadcast_to((HS, CQ, W)),
                        mybir.AluOpType.mult)
                    pool_prods[(qp, t)] = pr

            issue_pool_quarter(0)

            def emit_evac(psum_t, qp, j, via_pool=False):
                pa, pb = psum_t
                src_ap = pa[:, j:j + 1, :] if j < 3 else pb[:, 0:1, :]
                out_sb = opool.tile([HS, 2, W], BF16, tag="osb",
                                    name=f"osb{qp}_{j}")
                nc.scalar.copy(
                    out=out_sb.rearrange("p a b -> p (a b)"),
                    in_=src_ap.rearrange("p a b -> p (a b)"))
                eng = nc.gpsimd if via_pool else nc.sync
                eng.dma_start(
                    out=out_d[:, qp * CQ + 2 * j:qp * CQ + 2 * j + 2, :],
                    in_=out_sb)

            pool_pair_prods = {}

            def emit_quarter(qp, psum_t, c0, cn, j0, ph_ref, emit_ph,
                             pool_pairs=()):
                """One pass over the schedule for channels [c0, c0+cn) of
                quarter qp, accumulating into psum banks [j0, j0+cn/2)."""
                cq0 = qp * CQ
                sched = SCHED_Q[qp]
                nb = cn // 2
                pa, pb = psum_t

                def ps_ap(j):
                    return pa[:, j:j + 1, :] if j < 3 else pb[:, 0:1, :]

                first = True
                for si, (kind, pay) in enumerate(sched):
                    last = si == len(sched) - 1
                    if kind == 'pair' and si in pool_pairs:
                        ta, tb = pay
                        dp2 = pool_pair_prods.pop(si)
                        for k, t in enumerate((ta, tb)):
                            for j in range(nb):
                                nc.tensor.matmul(
                                    ps_ap(j0 + j),
                                    s_ap(t // K),
                                    dp2[:, k, 2 * j:2 * j + 2, :],
                                    start=first, stop=(last and k == 1))
                            first = False
                        if si == HALO_AFTER_Q[qp]:
                            raise AssertionError("halo slot can't be pooled")
                        continue
                    if kind == 'pool':
                        prod = pool_prods[(qp, pay)]
                        for j in range(nb):
                            nc.tensor.matmul(
                                ps_ap(j0 + j),
                                s_ap(pay // K),
                                prod[:, c0 + 2 * j:c0 + 2 * j + 2, :],
                                start=first, stop=last)
                        first = False
                    elif kind == 'single':
                        t = pay
                        par = (t % K) % 2
                        off = t % K - par
                        dps_f = dpool.tile([HS, 2, CQ, W], BF16, tag="dp",
                                           name="dps_f")
                        dps = dps_f[:, 0, 0:cn, :]
                        nc.vector.tensor_tensor(
                            dps,
                            faq[(par, qp)][:, c0:c0 + cn, off:off + W],
                            ks_ap(t).broadcast_to((HS, cn, W)),
                            mybir.AluOpType.mult)
                        for j in range(nb):
                            nc.tensor.matmul(
                                ps_ap(j0 + j),
                                s_ap(t // K),
                                dps[:, 2 * j:2 * j + 2, :],
                                start=first, stop=False)
                        first = False
                    else:
                        ta, tb = pay
                        dp2_f = dpool.tile([HS, 2, CQ, W], BF16, tag="dp",
                                           name="dp2_f")
                        dp2 = dp2_f[:, :, 0:cn, :]
                        in0, in1, _ = _pair_aps(faq[((ta % K) % 2, qp)],
                                                ks_ap(ta), ta, tb, c0, cn)
                        nc.vector.tensor_tensor(dp2, in0, in1,
                                                mybir.AluOpType.mult)
                        for k, t in enumerate((ta, tb)):
                            for j in range(nb):
                                nc.tensor.matmul(
                                    ps_ap(j0 + j),
                                    s_ap(t // K),
                                    dp2[:, k, 2 * j:2 * j + 2, :],
                                    start=first, stop=(last and k == 1))
                            first = False
                    if si == HALO_AFTER_Q[qp]:
                        if emit_ph:
                            # halo products double-packed: partitions [0:51]
                            # carry c0-3, [64:115] carry c4-7 -> half the
                            # DVE free size per quarter
                            ph_ref[0] = hpool.tile([NH2, 4, W], BF16,
                                                   tag="ph", name="ph_q")
                            nc.vector.tensor_tensor(
                                ph_ref[0], fhs_t[:, 4 * qp:4 * qp + 4, :],
                                kh_t.broadcast_to((NH2, 4, W)),
                                mybir.AluOpType.mult)
                        for j in range(nb):
                            jj = j0 + j
                            half = 0 if jj < 2 else 1
                            slot = (2 * jj) % 4
                            nc.tensor.matmul(
                                ps_ap(jj),
                                shal_t[:, half * HS:(half + 1) * HS],
                                ph_ref[0][:, slot:slot + 2, :],
                                start=False, stop=False)

            for qp in range(NQ):
                if qp + 1 < NQ:
                    issue_pool_quarter(qp + 1)
                psum_a = qpool.tile([HS, 3, 512], F32, tag="psA")
                psum_b = qpool.tile([HS, 1, 512], F32, tag="psB")
                psum_t = (psum_a, psum_b)
                ph_ref = [None]
                if qp < NQ - 1:
                    emit_quarter(qp, psum_t, 0, CQ, 0, ph_ref, True)
                    for j in range(4):
                        emit_evac(psum_t, qp, j)
                else:
                    # final quarter: 6+2 channel split so banks 0-2 close and
                    # drain while the small bank-3 group finishes -> short tail
                    emit_quarter(qp, psum_t, 0, 6, 0, ph_ref, True)
                    for j in range(3):
                        emit_evac(psum_t, qp, j)
                    emit_quarter(qp, psum_t, 6, 2, 3, ph_ref, False)
                    emit_evac(psum_t, qp, 3, via_pool=True)
    if not nc.is_finalized():
        nc.finalize()
    return nc


def _get_nc():
    if "nc" not in _NC_CACHE:
        _NC_CACHE["nc"] = _build_nc()
    return _NC_CACHE["nc"]


def _prep_inputs(feat, kernel, bias):
    bf = ml_dtypes.bfloat16
    ft = np.ascontiguousarray(feat.transpose(0, 1, 3, 2))   # [B, H, C, W]
    fp = np.zeros((B, H + 4, C, W + 6), np.float32)
    fp[:, 2:H + 2, :, 2:W + 2] = ft
    fpb = fp.astype(bf)
    kt = np.ascontiguousarray(kernel.transpose(0, 1, 3, 2))  # [B, H, KK, W]
    kp = np.zeros((B, H + 8, KK, W), np.float32)
    kp[:, 4:H + 4] = kt
    kpb = kp.astype(bf)
    biasb = bias.astype(bf)

    # stationaries (shared by all cores)
    sst = np.zeros((HS, K, HS), bf)
    for di in range(K):
        for po in range(HS - di):
            sst[po + di, di, po] = 1
    shal = np.zeros((NHALO, HS), bf)
    halo_rows = []          # (r, di, dj, q)
    r = 0
    for di in range(1, K):
        for q in range(di):
            for dj in range(K):
                halo_rows.append((r, di, dj, q))
                shal[r, HS + q - di] = 1
                r += 1
    shal[50, :] = 1         # bias row hits every output row
    # double-packed halo stationaries: A-half rows [0:51] (c0-3), B-half
    # rows [64:115] (c4-7); the other half is zero in each stationary
    shal2 = np.zeros((NH2, 2 * HS), bf)
    shal2[0:NHALO, 0:HS] = shal
    shal2[64:64 + NHALO, HS:2 * HS] = shal
    sst2 = np.ascontiguousarray(sst.reshape(HS, K * HS))

    in_maps = []
    for core in range(8):
        b, hh = core // 2, core % 2
        h0 = hh * HS
        fa0 = np.ascontiguousarray(fpb[b, h0:h0 + HS, :, 0:W + 4])
        fa1 = np.ascontiguousarray(fpb[b, h0:h0 + HS, :, 1:W + 5])
        ks = np.empty((HS, KK, W), bf)
        for t in range(KK):
            di, dj = t // K, t % K
            ks[:, di * K + DJ_ROW[dj], :] = \
                kpb[b, 4 + h0 - di:4 + h0 - di + HS, t, :]
        fhs = np.empty((NHALO, C, W), bf)
        kh = np.empty((NHALO, 1, W), bf)
        for (rr, di, dj, q) in halo_rows:
            fhs[rr, :, :] = fpb[b, h0 + HS + q, :, dj:dj + W]
            kh[rr, 0, :] = kpb[b, 4 + h0 + HS + q - di, di * K + dj, :]
        fhs[50, :, :] = np.broadcast_to(biasb[:, None], (C, W))
        kh[50, 0, :] = bf(1.0)
        fhs2 = np.zeros((NH2, 16, W), bf)
        kh2 = np.zeros((NH2, 1, W), bf)
        for q4 in range(NQ):
            fhs2[0:NHALO, 4 * q4:4 * q4 + 4, :] = \
                fhs[:, CQ * q4:CQ * q4 + 4, :]
            fhs2[64:64 + NHALO, 4 * q4:4 * q4 + 4, :] = \
                fhs[:, CQ * q4 + 4:CQ * q4 + 8, :]
        kh2[0:NHALO] = kh
        kh2[64:64 + NHALO] = kh
        in_maps.append({
            "fa0": fa0, "fa1": fa1,
            "ks": np.ascontiguousarray(ks),
            "sst": sst2, "shal": shal2,
            "fhs": np.ascontiguousarray(fhs2),
            "kh": np.ascontiguousarray(kh2),
        })
    return in_maps


def _run(feat, kernel, bias, **run_kwargs):
    nc = _get_nc()
    in_maps = _prep_inputs(feat, kernel, bias)
    res = run_bass_kernel_spmd(nc, in_maps, core_ids=list(range(8)),
                               **run_kwargs)
    out = np.empty((B, H, C, W), np.float32)
    for core in range(8):
        b, hh = core // 2, core % 2
        out[b, hh * HS:(hh + 1) * HS] = res.results[core]["out"].astype(
            np.float32)
    return np.ascontiguousarray(out.transpose(0, 1, 3, 2)), res


def kernel(feat, kernel, bias):
    out, _ = _run(np.asarray(feat, np.float32), np.asarray(kernel, np.float32),
                  np.asarray(bias, np.float32))
    return out


# revision 64
# speedup vs baseline: 1.0053x; 1.0053x over previous
"""Per-pixel predicted 5x5 conv (KPN-style) on 8 trn2 cores.

Sharding: data-parallel over (batch x H-half) = 8 shards of 128 output rows.

Design (v3):
  - feat loaded ONCE per core as 2 w-parity copies (4B-aligned stride-1 runs
    keep DVE tensor_tensor in 2x_1P bf16 mode for every tap's w-shift).
  - kern rows pre-shifted HOST-side per tap: KS[p, t] = kern[h0 + p - di(t), t],
    so prod_t[p] = featA[p] * KS[t, p] pairs the right kernel row with the
    right (h-shifted) feat row.
  - h-shift happens inside the PE accumulate via 5 shifted-identity
    stationaries S_di[p, po] = (p == po + di), masked to valid po < 128 - di.
    Stationary swaps cost nothing extra; PE streams each tap product once.
  - bottom-halo rows (po >= 128 - di) + bias are handled by ONE extra stacked
    matmul per psum bank: 50 (tap, halo-row) products + 1 bias row with a
    [51, 128] scatter stationary; the products are computed on DVE from
    host-prepped shifted operands.
  - 5 of the 25 tap products run on GPSIMD (Pool) to unload DVE.
  - DVE tap products are fused in pairs sharing di (same PE stationary) via
    custom overlapping access patterns -> half the DVE instruction count.
  - PSUM is split into a 3-bank and a 1-bank tile so the final quarter's
    small bank-3 group can close and drain independently (whole-tile psum
    dependency tracking otherwise serializes the tail).
"""

import sys

for p in ("/opt/pypackages", "/opt/trn_rl_repo"):
    if p not in sys.path:
        sys.path.insert(0, p)

import numpy as np
import ml_dtypes

import bass_rust
import concourse.bass as bass
import concourse.mybir as mybir
from concourse import bacc, tile
from concourse.bass_utils import run_bass_kernel_spmd

B, H, W, C, KK, K = 4, 256, 256, 32, 25, 5
HS = H // 2          # 128 output rows per core
CQ = 8               # channels per PSUM chunk (4 banks of 512 fp32)
NQ = C // CQ
NHALO = 51           # 50 (di,q<di,dj) fixup rows + 1 bias row
NH2 = 115            # halo rows split into two c-halves: [0:51] c0-3, [64:115] c4-7
POOL_TAPS = (2, 7, 12, 17, 22)   # dj=2 taps -> GPSIMD
BF16 = mybir.dt.bfloat16
F32 = mybir.dt.float32

# schedule items: ('pair', (ta, tb)) share di and parity; ('single', t);
# ('pool', t) is a product computed on GPSIMD.
_PAIRS0 = [(5 * di + 0, 5 * di + 4) for di in range(K)]   # dj 0,4 (parity 0)
_PAIRS1 = [(5 * di + 1, 5 * di + 3) for di in range(K)]   # dj 1,3 (parity 1)
SCHED_A = [('pair', _PAIRS0[0]), ('pool', 2),
           ('pair', _PAIRS0[1]), ('pool', 7),
           ('pair', _PAIRS0[2]), ('pool', 12),
           ('pair', _PAIRS0[3]), ('pool', 17),
           ('pair', _PAIRS0[4]),
           ('pair', _PAIRS1[0]), ('pool', 22),
           ('pair', _PAIRS1[1]), ('pair', _PAIRS1[2]),
           ('pair', _PAIRS1[3]), ('pair', _PAIRS1[4])]
# q0 variant with a 6th pool tap (14): q0's pools-late slots have the most
# deadline slack (PE p-state slow start + no competing prefetch)
SCHED_Z6 = [('single', 0), ('single', 4),
            ('pair', _PAIRS0[1]), ('single', 10),
            ('pool', 2),
            ('pair', _PAIRS0[3]), ('pair', _PAIRS0[4]),
            ('pool', 7),
            ('pair', _PAIRS1[0]), ('pair', _PAIRS1[1]),
            ('pool', 12),
            ('pair', _PAIRS1[2]), ('pair', _PAIRS1[3]),
            ('pool', 17),
            ('pool', 14),
            ('pair', _PAIRS1[4]),
            ('pool', 22)]
# pools-late for q1+ too: q0's extra pool op shifts the whole pool queue
SCHED_L = [('pair', _PAIRS0[0]), ('pair', _PAIRS0[1]), ('pair', _PAIRS0[2]),
           ('pool', 2),
           ('pair', _PAIRS0[3]), ('pair', _PAIRS0[4]),
           ('pool', 7),
           ('pair', _PAIRS1[0]), ('pair', _PAIRS1[1]),
           ('pool', 12),
           ('pair', _PAIRS1[2]), ('pair', _PAIRS1[3]),
           ('pool', 17),
           ('pair', _PAIRS1[4]),
           ('pool', 22)]
# q3 variant: tap 22 runs as a DVE single mid-quarter instead of the pool's
# last op — its 92.76us pool arrival was the final binding PE stall; the
# round-7 halo-repack slack lets DVE absorb it
SCHED_L3 = [('pair', _PAIRS0[0]), ('pair', _PAIRS0[1]), ('pair', _PAIRS0[2]),
            ('pool', 2),
            ('pair', _PAIRS0[3]), ('single', 22),
            ('pair', _PAIRS0[4]),
            ('pool', 7),
            ('pair', _PAIRS1[0]), ('pair', _PAIRS1[1]),
            ('pool', 12),
            ('pair', _PAIRS1[2]), ('pair', _PAIRS1[3]),
            ('pool', 17),
            ('pair', _PAIRS1[4])]
POOL_TAPS_Q = {0: (2, 7, 12, 17, 14, 22), 1: (2, 7, 12, 17, 22),
               2: (2, 7, 12, 17, 22), 3: (2, 7, 12, 17)}
SCHED_Q = {0: SCHED_Z6, 1: SCHED_L, 2: SCHED_L, 3: SCHED_L3}
# within each di-chunk, kernel rows are host-reordered [dj0, dj4, dj1, dj3,
# dj2] so both pair kinds read adjacent rows (stride W) and the pair-critical
# rows load first
DJ_ROW = {0: 0, 4: 1, 1: 2, 3: 3, 2: 4}
HALO_AFTER_Q = {0: 10, 1: 10, 2: 10, 3: 10}  # halo matmul position per quarter

_NC_CACHE = {}


def _pair_aps(fa_t, kb, ta, tb, c0=0, cn=CQ):
    """Overlapping-window APs computing both taps of a pair in one TT op,
    for channels [c0, c0+cn) of the quarter.

    in0[p, k, c, w] = fa_t[p, c0+c, off + k*step + w]  (k = 0/1 -> tap a/b)
    in1[p, k, c, w] = kb[p, k-th adjacent kernel row, w]  (c broadcast)
    """
    dja, djb = ta % K, tb % K
    par = dja % 2
    off = dja - par
    step = djb - dja                     # elements between the two w-shifts
    base = fa_t[:, c0:c0 + cn, off:off + W]
    a0 = base.ap
    in0 = bass.AP(tensor=base.tensor, offset=base.offset,
                  ap=bass_rust.VecI64Pair(
                      [list(a0[0]), [step, 2], list(a0[1]), list(a0[2])]))
    k0 = kb.ap
    in1 = bass.AP(tensor=kb.tensor, offset=kb.offset,
                  ap=bass_rust.VecI64Pair(
                      [list(k0[0]), [W, 2], [0, cn], list(k0[2])]))
    return in0, in1, par


def _build_nc():
    nc = bacc.Bacc(None, target_bir_lowering=False)
    fa0_d = nc.dram_tensor("fa0", [HS, C, W + 4], BF16, kind="ExternalInput")
    fa1_d = nc.dram_tensor("fa1", [HS, C, W + 4], BF16, kind="ExternalInput")
    ks_d = nc.dram_tensor("ks", [HS, KK, W], BF16, kind="ExternalInput")
    sst_d = nc.dram_tensor("sst", [HS, K * HS], BF16, kind="ExternalInput")
    shal_d = nc.dram_tensor("shal", [NH2, 2 * HS], BF16, kind="ExternalInput")
    fhs_d = nc.dram_tensor("fhs", [NH2, 16, W], BF16, kind="ExternalInput")
    kh_d = nc.dram_tensor("kh", [NH2, 1, W], BF16, kind="ExternalInput")
    out_d = nc.dram_tensor("out", [HS, C, W], BF16, kind="ExternalOutput")

    with tile.TileContext(nc) as tc:
        with tc.tile_pool(name="const", bufs=1) as cpool, \
             tc.tile_pool(name="dprod", bufs=6) as dpool, \
             tc.tile_pool(name="pprod", bufs=2) as ppool, \
             tc.tile_pool(name="hprod", bufs=2) as hpool, \
             tc.tile_pool(name="osb", bufs=4) as opool, \
             tc.tile_pool(name="psum", bufs=2, space="PSUM") as qpool:
            ks_a, ks_b = [], []
            for di in range(K):
                ka = cpool.tile([HS, 2, W], BF16, tag=f"ksa{di}",
                                name=f"ksa{di}")
                kb = cpool.tile([HS, 3, W], BF16, tag=f"ksb{di}",
                                name=f"ksb{di}")
                ks_a.append(ka)
                ks_b.append(kb)
            faq = {}
            for q in range(NQ):
                for par in range(2):
                    t_ = cpool.tile([HS, CQ, W + 4], BF16,
                                    tag=f"fa{par}q{q}", name=f"fa{par}q{q}")
                    faq[(par, q)] = t_
            fa_d = {0: fa0_d, 1: fa1_d}

            def load_faq(par, q):
                nc.sync.dma_start(
                    out=faq[(par, q)],
                    in_=fa_d[par][:, q * CQ:(q + 1) * CQ, :])

            def load_ks(di, half):
                if half == 0:
                    nc.sync.dma_start(out=ks_a[di],
                                      in_=ks_d[:, di * K:di * K + 2, :])
                else:
                    nc.sync.dma_start(out=ks_b[di],
                                      in_=ks_d[:, di * K + 2:di * K + K, :])

            # DMA order: quarter-0 critical operands first
            load_faq(0, 0)
            load_ks(0, 0)
            load_ks(0, 1)
            load_ks(1, 0)
            sst_t = cpool.tile([HS, K * HS], BF16, tag="sst")
            nc.sync.dma_start(out=sst_t, in_=sst_d[:, :])
            load_faq(1, 0)
            load_ks(1, 1)
            for di in range(2, K):
                load_ks(di, 0)
                load_ks(di, 1)
            load_faq(0, 1)
            load_faq(1, 1)
            fhs_t = cpool.tile([NH2, 16, W], BF16, tag="fhs")
            nc.sync.dma_start(out=fhs_t, in_=fhs_d[:, :, :])
            kh_t = cpool.tile([NH2, 1, W], BF16, tag="kh")
            nc.sync.dma_start(out=kh_t, in_=kh_d[:, :, :])
            shal_t = cpool.tile([NH2, 2 * HS], BF16, tag="shal")
            nc.sync.dma_start(out=shal_t, in_=shal_d[:, :])
            for q in range(2, NQ):
                load_faq(0, q)
                load_faq(1, q)

            def s_ap(di):
                return sst_t[:, di * HS:(di + 1) * HS]

            def ks_ap(t):
                di, dj = t // K, t % K
                row = DJ_ROW[dj]
                if row < 2:
                    return ks_a[di][:, row:row + 1, :]
                return ks_b[di][:, row - 2:row - 1, :]

            # pool products, prefetched one quarter ahead
            pool_prods = {}

            def issue_pool_quarter(qp):
                for t in POOL_TAPS_Q[qp]:
                    par = (t % K) % 2
                    off = t % K - par
                    pr = ppool.tile([HS, CQ, W], BF16, tag=f"pp{t}",
                                    name=f"pp{t}_{qp}")
                    nc.gpsimd.tensor_tensor(
                        pr,
                        faq[(par, qp)][:, :, off:off + W],
                        ks_ap(t).broadcast_to((HS, CQ, W)),
                        mybir.AluOpType.mult)
                    pool_prods[(qp, t)] = pr

            issue_pool_quarter(0)

            def emit_evac(psum_t, qp, j, via_pool=False):
                pa, pb = psum_t
                src_ap = pa[:, j:j + 1, :] if j < 3 else pb[:, 0:1, :]
                out_sb = opool.tile([HS, 2, W], BF16, tag="osb",
                                    name=f"osb{qp}_{j}")
                nc.scalar.copy(
                    out=out_sb.rearrange("p a b -> p (a b)"),
                    in_=src_ap.rearrange("p a b -> p (a b)"))
                eng = nc.gpsimd if via_pool else nc.sync
                eng.dma_start(
                    out=out_d[:, qp * CQ + 2 * j:qp * CQ + 2 * j + 2, :],
                    in_=out_sb)

            pool_pair_prods = {}

            def emit_quarter(qp, psum_t, c0, cn, j0, ph_ref, emit_ph,
                             pool_pairs=()):
                """One pass over the schedule for channels [c0, c0+cn) of
                quarter qp, accumulating into psum banks [j0, j0+cn/2)."""
                cq0 = qp * CQ
                sched = SCHED_Q[qp]
                nb = cn // 2
                pa, pb = psum_t

                def ps_ap(j):
                    return pa[:, j:j + 1, :] if j < 3 else pb[:, 0:1, :]

                first = True
                for si, (kind, pay) in enumerate(sched):
                    last = si == len(sched) - 1
                    if kind == 'pair' and si in pool_pairs:
                        ta, tb = pay
                        dp2 = pool_pair_prods.pop(si)
                        for k, t in enumerate((ta, tb)):
                            for j in range(nb):
                                nc.tensor.matmul(
                                    ps_ap(j0 + j),
                                    s_ap(t // K),
                                    dp2[:, k, 2 * j:2 * j + 2, :],
                                    start=first, stop=(last and k == 1))
                            first = False
                        if si == HALO_AFTER_Q[qp]:
                            raise AssertionError("halo slot can't be pooled")
                        continue
                    if kind == 'pool':
                        prod = pool_prods[(qp, pay)]
                        for j in range(nb):
                            nc.tensor.matmul(
                                ps_ap(j0 + j),
                                s_ap(pay // K),
                                prod[:, c0 + 2 * j:c0 + 2 * j + 2, :],
                                start=first, stop=last)
                        first = False
                    elif kind == 'single':
                        t = pay
                        par = (t % K) % 2
                        off = t % K - par
                        dps_f = dpool.tile([HS, 2, CQ, W], BF16, tag="dp",
                                           name="dps_f")
                        dps = dps_f[:, 0, 0:cn, :]
                        nc.vector.tensor_tensor(
                            dps,
                            faq[(par, qp)][:, c0:c0 + cn, off:off + W],
                            ks_ap(t).broadcast_to((HS, cn, W)),
                            mybir.AluOpType.mult)
                        for j in range(nb):
                            nc.tensor.matmul(
                                ps_ap(j0 + j),
                                s_ap(t // K),
                                dps[:, 2 * j:2 * j + 2, :],
                                start=first, stop=False)
                        first = False
                    else:
                        ta, tb = pay
                        dp2_f = dpool.tile([HS, 2, CQ, W], BF16, tag="dp",
                                           name="dp2_f")
                        dp2 = dp2_f[:, :, 0:cn, :]
                        in0, in1, _ = _pair_aps(faq[((ta % K) % 2, qp)],
                                                ks_ap(ta), ta, tb, c0, cn)
                        nc.vector.tensor_tensor(dp2, in0, in1,
                                                mybir.AluOpType.mult)
                        for k, t in enumerate((ta, tb)):
                            for j in range(nb):
                                nc.tensor.matmul(
                                    ps_ap(j0 + j),
                                    s_ap(t // K),
                                    dp2[:, k, 2 * j:2 * j + 2, :],
                                    start=first, stop=(last and k == 1))
                            first = False
                    if si == HALO_AFTER_Q[qp]:
                        if emit_ph:
                            # halo products double-packed: partitions [0:51]
                            # carry c0-3, [64:115] carry c4-7 -> half the
                            # DVE free size per quarter
                            ph_ref[0] = hpool.tile([NH2, 4, W], BF16,
                                                   tag="ph", name="ph_q")
                            nc.vector.tensor_tensor(
                                ph_ref[0], fhs_t[:, 4 * qp:4 * qp + 4, :],
                                kh_t.broadcast_to((NH2, 4, W)),
                                mybir.AluOpType.mult)
                        for j in range(nb):
                            jj = j0 + j
                            half = 0 if jj < 2 else 1
                            slot = (2 * jj) % 4
                            nc.tensor.matmul(
                                ps_ap(jj),
                                shal_t[:, half * HS:(half + 1) * HS],
                                ph_ref[0][:, slot:slot + 2, :],
                                start=False, stop=False)

            for qp in range(NQ):
                if qp + 1 < NQ:
                    issue_pool_quarter(qp + 1)
                psum_a = qpool.tile([HS, 3, 512], F32, tag="psA")
                psum_b = qpool.tile([HS, 1, 512], F32, tag="psB")
                psum_t = (psum_a, psum_b)
                ph_ref = [None]
                if qp < NQ - 1:
                    emit_quarter(qp, psum_t, 0, CQ, 0, ph_ref, True)
                    for j in range(4):
                        emit_evac(psum_t, qp, j)
                else:
                    # final quarter: 6+2 channel split so banks 0-2 close and
                    # drain while the small bank-3 group finishes -> short tail
                    emit_quarter(qp, psum_t, 0, 6, 0, ph_ref, True)
                    for j in range(3):
                        emit_evac(psum_t, qp, j)
                    emit_quarter(qp, psum_t, 6, 2, 3, ph_ref, False)
                    emit_evac(psum_t, qp, 3, via_pool=True)
    if not nc.is_finalized():
        nc.finalize()
    return nc


def _get_nc():
    if "nc" not in _NC_CACHE:
        _NC_CACHE["nc"] = _build_nc()
    return _NC_CACHE["nc"]


def _prep_inputs(feat, kernel, bias):
    bf = ml_dtypes.bfloat16
    ft = np.ascontiguousarray(feat.transpose(0, 1, 3, 2))   # [B, H, C, W]
    fp = np.zeros((B, H + 4, C, W + 6), np.float32)
    fp[:, 2:H + 2, :, 2:W + 2] = ft
    fpb = fp.astype(bf)
    kt = np.ascontiguousarray(kernel.transpose(0, 1, 3, 2))  # [B, H, KK, W]
    kp = np.zeros((B, H + 8, KK, W), np.float32)
    kp[:, 4:H + 4] = kt
    kpb = kp.astype(bf)
    biasb = bias.astype(bf)

    # stationaries (shared by all cores)
    sst = np.zeros((HS, K, HS), bf)
    for di in range(K):
        for po in range(HS - di):
            sst[po + di, di, po] = 1
    shal = np.zeros((NHALO, HS), bf)
    halo_rows = []          # (r, di, dj, q)
    r = 0
    for di in range(1, K):
        for q in range(di):
            for dj in range(K):
                halo_rows.append((r, di, dj, q))
                shal[r, HS + q - di] = 1
                r += 1
    shal[50, :] = 1         # bias row hits every output row
    # double-packed halo stationaries: A-half rows [0:51] (c0-3), B-half
    # rows [64:115] (c4-7); the other half is zero in each stationary
    shal2 = np.zeros((NH2, 2 * HS), bf)
    shal2[0:NHALO, 0:HS] = shal
    shal2[64:64 + NHALO, HS:2 * HS] = shal
    sst2 = np.ascontiguousarray(sst.reshape(HS, K * HS))

    in_maps = []
    for core in range(8):
        b, hh = core // 2, core % 2
        h0 = hh * HS
        fa0 = np.ascontiguousarray(fpb[b, h0:h0 + HS, :, 0:W + 4])
        fa1 = np.ascontiguousarray(fpb[b, h0:h0 + HS, :, 1:W + 5])
        ks = np.empty((HS, KK, W), bf)
        for t in range(KK):
            di, dj = t // K, t % K
            ks[:, di * K + DJ_ROW[dj], :] = \
                kpb[b, 4 + h0 - di:4 + h0 - di + HS, t, :]
        fhs = np.empty((NHALO, C, W), bf)
        kh = np.empty((NHALO, 1, W), bf)
        for (rr, di, dj, q) in halo_rows:
            fhs[rr, :, :] = fpb[b, h0 + HS + q, :, dj:dj + W]
            kh[rr, 0, :] = kpb[b, 4 + h0 + HS + q - di, di * K + dj, :]
        fhs[50, :, :] = np.broadcast_to(biasb[:, None], (C, W))
        kh[50, 0, :] = bf(1.0)
        fhs2 = np.zeros((NH2, 16, W), bf)
        kh2 = np.zeros((NH2, 1, W), bf)
        for q4 in range(NQ):
            fhs2[0:NHALO, 4 * q4:4 * q4 + 4, :] = \
                fhs[:, CQ * q4:CQ * q4 + 4, :]
            fhs2[64:64 + NHALO, 4 * q4:4 * q4 + 4, :] = \
                fhs[:, CQ * q4 + 4:CQ * q4 + 8, :]
        kh2[0:NHALO] = kh
        kh2[64:64 + NHALO] = kh
        in_maps.append({
            "fa0": fa0, "fa1": fa1,
            "ks": np.ascontiguousarray(ks),
            "sst": sst2, "shal": shal2,
            "fhs": np.ascontiguousarray(fhs2),
            "kh": np.ascontiguousarray(kh2),
        })
    return in_maps


def _run(feat, kernel, bias, **run_kwargs):
    nc = _get_nc()
    in_maps = _prep_inputs(feat, kernel, bias)
    res = run_bass_kernel_spmd(nc, in_maps, core_ids=list(range(8)),
                               **run_kwargs)
    out = np.empty((B, H, C, W), np.float32)
    for core in range(8):
        b, hh = core // 2, core % 2
        out[b, hh * HS:(hh + 1) * HS] = res.results[core]["out"].astype(
            np.float32)
    return np.ascontiguousarray(out.transpose(0, 1, 3, 2)), res


def kernel(feat, kernel, bias):
    out, _ = _run(np.asarray(feat, np.float32), np.asarray(kernel, np.float32),
                  np.asarray(bias, np.float32))
    return out


# revision 65
# speedup vs baseline: 1.0106x; 1.0052x over previous
"""Per-pixel predicted 5x5 conv (KPN-style) on 8 trn2 cores.

Sharding: data-parallel over (batch x H-half) = 8 shards of 128 output rows.

Design (v3):
  - feat loaded ONCE per core as 2 w-parity copies (4B-aligned stride-1 runs
    keep DVE tensor_tensor in 2x_1P bf16 mode for every tap's w-shift).
  - kern rows pre-shifted HOST-side per tap: KS[p, t] = kern[h0 + p - di(t), t],
    so prod_t[p] = featA[p] * KS[t, p] pairs the right kernel row with the
    right (h-shifted) feat row.
  - h-shift happens inside the PE accumulate via 5 shifted-identity
    stationaries S_di[p, po] = (p == po + di), masked to valid po < 128 - di.
    Stationary swaps cost nothing extra; PE streams each tap product once.
  - bottom-halo rows (po >= 128 - di) + bias are handled by ONE extra stacked
    matmul per psum bank: 50 (tap, halo-row) products + 1 bias row with a
    [51, 128] scatter stationary; the products are computed on DVE from
    host-prepped shifted operands.
  - 5 of the 25 tap products run on GPSIMD (Pool) to unload DVE.
  - DVE tap products are fused in pairs sharing di (same PE stationary) via
    custom overlapping access patterns -> half the DVE instruction count.
  - PSUM is split into a 3-bank and a 1-bank tile so the final quarter's
    small bank-3 group can close and drain independently (whole-tile psum
    dependency tracking otherwise serializes the tail).
"""

import sys

for p in ("/opt/pypackages", "/opt/trn_rl_repo"):
    if p not in sys.path:
        sys.path.insert(0, p)

import numpy as np
import ml_dtypes

import bass_rust
import concourse.bass as bass
import concourse.mybir as mybir
from concourse import bacc, tile
from concourse.bass_utils import run_bass_kernel_spmd

B, H, W, C, KK, K = 4, 256, 256, 32, 25, 5
HS = H // 2          # 128 output rows per core
CQ = 8               # channels per PSUM chunk (4 banks of 512 fp32)
NQ = C // CQ
NHALO = 51           # 50 (di,q<di,dj) fixup rows + 1 bias row
NH2 = 115            # halo rows split into two c-halves: [0:51] c0-3, [64:115] c4-7
POOL_TAPS = (2, 7, 12, 17, 22)   # dj=2 taps -> GPSIMD
BF16 = mybir.dt.bfloat16
F32 = mybir.dt.float32

# schedule items: ('pair', (ta, tb)) share di and parity; ('single', t);
# ('pool', t) is a product computed on GPSIMD.
_PAIRS0 = [(5 * di + 0, 5 * di + 4) for di in range(K)]   # dj 0,4 (parity 0)
_PAIRS1 = [(5 * di + 1, 5 * di + 3) for di in range(K)]   # dj 1,3 (parity 1)
SCHED_A = [('pair', _PAIRS0[0]), ('pool', 2),
           ('pair', _PAIRS0[1]), ('pool', 7),
           ('pair', _PAIRS0[2]), ('pool', 12),
           ('pair', _PAIRS0[3]), ('pool', 17),
           ('pair', _PAIRS0[4]),
           ('pair', _PAIRS1[0]), ('pool', 22),
           ('pair', _PAIRS1[1]), ('pair', _PAIRS1[2]),
           ('pair', _PAIRS1[3]), ('pair', _PAIRS1[4])]
# q0 variant with a 6th pool tap (14): q0's pools-late slots have the most
# deadline slack (PE p-state slow start + no competing prefetch)
SCHED_Z6 = [('single', 0), ('single', 4),
            ('pair', _PAIRS0[1]), ('single', 10),
            ('pool', 2),
            ('pair', _PAIRS0[3]), ('pair', _PAIRS0[4]),
            ('pool', 7),
            ('pair', _PAIRS1[0]), ('pair', _PAIRS1[1]),
            ('pool', 12),
            ('pair', _PAIRS1[2]), ('pair', _PAIRS1[3]),
            ('pool', 17),
            ('pool', 14),
            ('pair', _PAIRS1[4]),
            ('pool', 22)]
# pools-late for q1+ too: q0's extra pool op shifts the whole pool queue
SCHED_L = [('pair', _PAIRS0[0]), ('pair', _PAIRS0[1]), ('pair', _PAIRS0[2]),
           ('pool', 2),
           ('pair', _PAIRS0[3]), ('pair', _PAIRS0[4]),
           ('pool', 7),
           ('pair', _PAIRS1[0]), ('pair', _PAIRS1[1]),
           ('pool', 12),
           ('pair', _PAIRS1[2]), ('pair', _PAIRS1[3]),
           ('pool', 17),
           ('pair', _PAIRS1[4]),
           ('pool', 22)]
# q3 variant: tap 22 runs as a DVE single mid-quarter instead of the pool's
# last op — its 92.76us pool arrival was the final binding PE stall; the
# round-7 halo-repack slack lets DVE absorb it
SCHED_L3 = [('pair', _PAIRS0[0]), ('pair', _PAIRS0[1]), ('pair', _PAIRS0[2]),
            ('pool', 2),
            ('pair', _PAIRS0[3]), ('single', 22),
            ('pair', _PAIRS0[4]),
            ('pool', 7),
            ('pair', _PAIRS1[0]), ('pair', _PAIRS1[1]),
            ('pool', 12),
            ('pair', _PAIRS1[2]), ('pair', _PAIRS1[3]),
            ('pool', 17),
            ('pair', _PAIRS1[4])]
POOL_TAPS_Q = {0: (2, 7, 12, 17, 14, 22), 1: (2, 7, 12, 17, 22),
               2: (2, 7, 12, 17, 22), 3: (2, 7, 12, 17)}
SCHED_Q = {0: SCHED_Z6, 1: SCHED_L, 2: SCHED_L, 3: SCHED_L3}
# within each di-chunk, kernel rows are host-reordered [dj0, dj4, dj1, dj3,
# dj2] so both pair kinds read adjacent rows (stride W) and the pair-critical
# rows load first
DJ_ROW = {0: 0, 4: 1, 1: 2, 3: 3, 2: 4}
HALO_AFTER_Q = {0: 10, 1: 10, 2: 10, 3: 10}  # halo matmul position per quarter

_NC_CACHE = {}


def _pair_aps(fa_t, kb, ta, tb, c0=0, cn=CQ):
    """Overlapping-window APs computing both taps of a pair in one TT op,
    for channels [c0, c0+cn) of the quarter.

    in0[p, k, c, w] = fa_t[p, c0+c, off + k*step + w]  (k = 0/1 -> tap a/b)
    in1[p, k, c, w] = kb[p, k-th adjacent kernel row, w]  (c broadcast)
    """
    dja, djb = ta % K, tb % K
    par = dja % 2
    off = dja - par
    step = djb - dja                     # elements between the two w-shifts
    base = fa_t[:, c0:c0 + cn, off:off + W]
    a0 = base.ap
    in0 = bass.AP(tensor=base.tensor, offset=base.offset,
                  ap=bass_rust.VecI64Pair(
                      [list(a0[0]), [step, 2], list(a0[1]), list(a0[2])]))
    k0 = kb.ap
    in1 = bass.AP(tensor=kb.tensor, offset=kb.offset,
                  ap=bass_rust.VecI64Pair(
                      [list(k0[0]), [W, 2], [0, cn], list(k0[2])]))
    return in0, in1, par


def _build_nc():
    nc = bacc.Bacc(None, target_bir_lowering=False)
    fa0_d = nc.dram_tensor("fa0", [HS, C, W + 4], BF16, kind="ExternalInput")
    fa1_d = nc.dram_tensor("fa1", [HS, C, W + 4], BF16, kind="ExternalInput")
    ks_d = nc.dram_tensor("ks", [HS, KK, W], BF16, kind="ExternalInput")
    sst_d = nc.dram_tensor("sst", [HS, K * HS], BF16, kind="ExternalInput")
    shal_d = nc.dram_tensor("shal", [NH2, 2 * HS], BF16, kind="ExternalInput")
    fhs_d = nc.dram_tensor("fhs", [NH2, 16, W], BF16, kind="ExternalInput")
    kh_d = nc.dram_tensor("kh", [NH2, 1, W], BF16, kind="ExternalInput")
    out_d = nc.dram_tensor("out", [HS, C, W], BF16, kind="ExternalOutput")

    with tile.TileContext(nc) as tc:
        with tc.tile_pool(name="const", bufs=1) as cpool, \
             tc.tile_pool(name="dprod", bufs=6) as dpool, \
             tc.tile_pool(name="pprod", bufs=2) as ppool, \
             tc.tile_pool(name="hprod", bufs=2) as hpool, \
             tc.tile_pool(name="osb", bufs=4) as opool, \
             tc.tile_pool(name="psum", bufs=2, space="PSUM") as qpool:
            ks_a, ks_b = [], []
            for di in range(K):
                ka = cpool.tile([HS, 2, W], BF16, tag=f"ksa{di}",
                                name=f"ksa{di}")
                kb = cpool.tile([HS, 3, W], BF16, tag=f"ksb{di}",
                                name=f"ksb{di}")
                ks_a.append(ka)
                ks_b.append(kb)
            faq = {}
            for q in range(NQ):
                for par in range(2):
                    t_ = cpool.tile([HS, CQ, W + 4], BF16,
                                    tag=f"fa{par}q{q}", name=f"fa{par}q{q}")
                    faq[(par, q)] = t_
            fa_d = {0: fa0_d, 1: fa1_d}

            def load_faq(par, q):
                nc.sync.dma_start(
                    out=faq[(par, q)],
                    in_=fa_d[par][:, q * CQ:(q + 1) * CQ, :])

            def load_ks(di, half):
                if half == 0:
                    nc.sync.dma_start(out=ks_a[di],
                                      in_=ks_d[:, di * K:di * K + 2, :])
                else:
                    nc.sync.dma_start(out=ks_b[di],
                                      in_=ks_d[:, di * K + 2:di * K + K, :])

            # DMA order: quarter-0 critical operands first
            load_faq(0, 0)
            load_ks(0, 0)
            load_ks(0, 1)
            load_ks(1, 0)
            sst_t = cpool.tile([HS, K * HS], BF16, tag="sst")
            nc.sync.dma_start(out=sst_t, in_=sst_d[:, :])
            load_faq(1, 0)
            load_ks(1, 1)
            for di in range(2, K):
                load_ks(di, 0)
                load_ks(di, 1)
            load_faq(0, 1)
            load_faq(1, 1)
            fhs_t = cpool.tile([NH2, 16, W], BF16, tag="fhs")
            nc.sync.dma_start(out=fhs_t, in_=fhs_d[:, :, :])
            kh_t = cpool.tile([NH2, 1, W], BF16, tag="kh")
            nc.sync.dma_start(out=kh_t, in_=kh_d[:, :, :])
            shal_t = cpool.tile([NH2, 2 * HS], BF16, tag="shal")
            nc.sync.dma_start(out=shal_t, in_=shal_d[:, :])
            for q in range(2, NQ):
                load_faq(0, q)
                load_faq(1, q)

            def s_ap(di):
                return sst_t[:, di * HS:(di + 1) * HS]

            def ks_ap(t):
                di, dj = t // K, t % K
                row = DJ_ROW[dj]
                if row < 2:
                    return ks_a[di][:, row:row + 1, :]
                return ks_b[di][:, row - 2:row - 1, :]

            # pool products, prefetched one quarter ahead
            pool_prods = {}

            def issue_pool_quarter(qp):
                for t in POOL_TAPS_Q[qp]:
                    par = (t % K) % 2
                    off = t % K - par
                    pr = ppool.tile([HS, CQ, W], BF16, tag=f"pp{t}",
                                    name=f"pp{t}_{qp}")
                    nc.gpsimd.tensor_tensor(
                        pr,
                        faq[(par, qp)][:, :, off:off + W],
                        ks_ap(t).broadcast_to((HS, CQ, W)),
                        mybir.AluOpType.mult)
                    pool_prods[(qp, t)] = pr

            issue_pool_quarter(0)

            def emit_evac(psum_t, qp, j, via_pool=False):
                pa, pb = psum_t
                src_ap = pa[:, j:j + 1, :] if j < 3 else pb[:, 0:1, :]
                out_sb = opool.tile([HS, 2, W], BF16, tag="osb",
                                    name=f"osb{qp}_{j}")
                nc.scalar.copy(
                    out=out_sb.rearrange("p a b -> p (a b)"),
                    in_=src_ap.rearrange("p a b -> p (a b)"))
                eng = nc.gpsimd if via_pool else nc.sync
                eng.dma_start(
                    out=out_d[:, qp * CQ + 2 * j:qp * CQ + 2 * j + 2, :],
                    in_=out_sb)

            pool_pair_prods = {}

            def emit_quarter(qp, psum_t, c0, cn, j0, ph_ref, emit_ph,
                             pool_pairs=()):
                """One pass over the schedule for channels [c0, c0+cn) of
                quarter qp, accumulating into psum banks [j0, j0+cn/2)."""
                cq0 = qp * CQ
                sched = SCHED_Q[qp]
                nb = cn // 2
                pa, pb = psum_t

                def ps_ap(j):
                    return pa[:, j:j + 1, :] if j < 3 else pb[:, 0:1, :]

                first = True
                for si, (kind, pay) in enumerate(sched):
                    last = si == len(sched) - 1
                    if kind == 'pair' and si in pool_pairs:
                        ta, tb = pay
                        dp2 = pool_pair_prods.pop(si)
                        for k, t in enumerate((ta, tb)):
                            for j in range(nb):
                                nc.tensor.matmul(
                                    ps_ap(j0 + j),
                                    s_ap(t // K),
                                    dp2[:, k, 2 * j:2 * j + 2, :],
                                    start=first, stop=(last and k == 1))
                            first = False
                        if si == HALO_AFTER_Q[qp]:
                            raise AssertionError("halo slot can't be pooled")
                        continue
                    if kind == 'pool':
                        prod = pool_prods[(qp, pay)]
                        for j in range(nb):
                            nc.tensor.matmul(
                                ps_ap(j0 + j),
                                s_ap(pay // K),
                                prod[:, c0 + 2 * j:c0 + 2 * j + 2, :],
                                start=first, stop=last)
                        first = False
                    elif kind == 'single':
                        t = pay
                        par = (t % K) % 2
                        off = t % K - par
                        dps_f = dpool.tile([HS, 2, CQ, W], BF16, tag="dp",
                                           name="dps_f")
                        dps = dps_f[:, 0, 0:cn, :]
                        nc.vector.tensor_tensor(
                            dps,
                            faq[(par, qp)][:, c0:c0 + cn, off:off + W],
                            ks_ap(t).broadcast_to((HS, cn, W)),
                            mybir.AluOpType.mult)
                        for j in range(nb):
                            nc.tensor.matmul(
                                ps_ap(j0 + j),
                                s_ap(t // K),
                                dps[:, 2 * j:2 * j + 2, :],
                                start=first, stop=False)
                        first = False
                    elif kind == 'pair' and qp == 0 and si == 0:
                        # first product in two SEPARATE tiles (6c + 2c): PE's
                        # first matmul starts off the 6c piece ~0.5us earlier
                        # (separate tiles -> separate deps; one tile would
                        # serialize on whole-tile tracking)
                        ta, tb = pay
                        dpa = dpool.tile([HS, 2, 6, W], BF16, tag="dpa",
                                         name="dpa")
                        dpb = dpool.tile([HS, 2, 2, W], BF16, tag="dpb",
                                         name="dpb")
                        for t_, cc0, ccn in ((dpa, 0, 6), (dpb, 6, 2)):
                            i0_, i1_, _ = _pair_aps(faq[((ta % K) % 2, qp)],
                                                    ks_ap(ta), ta, tb,
                                                    cc0, ccn)
                            nc.vector.tensor_tensor(t_, i0_, i1_,
                                                    mybir.AluOpType.mult)
                        for k, t in enumerate((ta, tb)):
                            for j in range(3):
                                nc.tensor.matmul(
                                    ps_ap(j), s_ap(t // K),
                                    dpa[:, k, 2 * j:2 * j + 2, :],
                                    start=first, stop=False)
                            nc.tensor.matmul(
                                ps_ap(3), s_ap(t // K),
                                dpb[:, k, :, :],
                                start=first, stop=False)
                            first = False
                    else:
                        ta, tb = pay
                        dp2_f = dpool.tile([HS, 2, CQ, W], BF16, tag="dp",
                                           name="dp2_f")
                        dp2 = dp2_f[:, :, 0:cn, :]
                        in0, in1, _ = _pair_aps(faq[((ta % K) % 2, qp)],
                                                ks_ap(ta), ta, tb, c0, cn)
                        nc.vector.tensor_tensor(dp2, in0, in1,
                                                mybir.AluOpType.mult)
                        for k, t in enumerate((ta, tb)):
                            for j in range(nb):
                                nc.tensor.matmul(
                                    ps_ap(j0 + j),
                                    s_ap(t // K),
                                    dp2[:, k, 2 * j:2 * j + 2, :],
                                    start=first, stop=(last and k == 1))
                            first = False
                    if si == HALO_AFTER_Q[qp]:
                        if emit_ph:
                            # halo products double-packed: partitions [0:51]
                            # carry c0-3, [64:115] carry c4-7 -> half the
                            # DVE free size per quarter
                            ph_ref[0] = hpool.tile([NH2, 4, W], BF16,
                                                   tag="ph", name="ph_q")
                            nc.vector.tensor_tensor(
                                ph_ref[0], fhs_t[:, 4 * qp:4 * qp + 4, :],
                                kh_t.broadcast_to((NH2, 4, W)),
                                mybir.AluOpType.mult)
                        for j in range(nb):
                            jj = j0 + j
                            half = 0 if jj < 2 else 1
                            slot = (2 * jj) % 4
                            nc.tensor.matmul(
                                ps_ap(jj),
                                shal_t[:, half * HS:(half + 1) * HS],
                                ph_ref[0][:, slot:slot + 2, :],
                                start=False, stop=False)

            for qp in range(NQ):
                if qp + 1 < NQ:
                    issue_pool_quarter(qp + 1)
                psum_a = qpool.tile([HS, 3, 512], F32, tag="psA")
                psum_b = qpool.tile([HS, 1, 512], F32, tag="psB")
                psum_t = (psum_a, psum_b)
                ph_ref = [None]
                if qp < NQ - 1:
                    emit_quarter(qp, psum_t, 0, CQ, 0, ph_ref, True)
                    for j in range(4):
                        emit_evac(psum_t, qp, j)
                else:
                    # final quarter: 6+2 channel split so banks 0-2 close and
                    # drain while the small bank-3 group finishes -> short tail
                    emit_quarter(qp, psum_t, 0, 6, 0, ph_ref, True)
                    for j in range(3):
                        emit_evac(psum_t, qp, j)
                    emit_quarter(qp, psum_t, 6, 2, 3, ph_ref, False)
                    emit_evac(psum_t, qp, 3, via_pool=True)
    if not nc.is_finalized():
        nc.finalize()
    return nc


def _get_nc():
    if "nc" not in _NC_CACHE:
        _NC_CACHE["nc"] = _build_nc()
    return _NC_CACHE["nc"]


def _prep_inputs(feat, kernel, bias):
    bf = ml_dtypes.bfloat16
    ft = np.ascontiguousarray(feat.transpose(0, 1, 3, 2))   # [B, H, C, W]
    fp = np.zeros((B, H + 4, C, W + 6), np.float32)
    fp[:, 2:H + 2, :, 2:W + 2] = ft
    fpb = fp.astype(bf)
    kt = np.ascontiguousarray(kernel.transpose(0, 1, 3, 2))  # [B, H, KK, W]
    kp = np.zeros((B, H + 8, KK, W), np.float32)
    kp[:, 4:H + 4] = kt
    kpb = kp.astype(bf)
    biasb = bias.astype(bf)

    # stationaries (shared by all cores)
    sst = np.zeros((HS, K, HS), bf)
    for di in range(K):
        for po in range(HS - di):
            sst[po + di, di, po] = 1
    shal = np.zeros((NHALO, HS), bf)
    halo_rows = []          # (r, di, dj, q)
    r = 0
    for di in range(1, K):
        for q in range(di):
            for dj in range(K):
                halo_rows.append((r, di, dj, q))
                shal[r, HS + q - di] = 1
                r += 1
    shal[50, :] = 1         # bias row hits every output row
    # double-packed halo stationaries: A-half rows [0:51] (c0-3), B-half
    # rows [64:115] (c4-7); the other half is zero in each stationary
    shal2 = np.zeros((NH2, 2 * HS), bf)
    shal2[0:NHALO, 0:HS] = shal
    shal2[64:64 + NHALO, HS:2 * HS] = shal
    sst2 = np.ascontiguousarray(sst.reshape(HS, K * HS))

    in_maps = []
    for core in range(8):
        b, hh = core // 2, core % 2
        h0 = hh * HS
        fa0 = np.ascontiguousarray(fpb[b, h0:h0 + HS, :, 0:W + 4])
        fa1 = np.ascontiguousarray(fpb[b, h0:h0 + HS, :, 1:W + 5])
        ks = np.empty((HS, KK, W), bf)
        for t in range(KK):
            di, dj = t // K, t % K
            ks[:, di * K + DJ_ROW[dj], :] = \
                kpb[b, 4 + h0 - di:4 + h0 - di + HS, t, :]
        fhs = np.empty((NHALO, C, W), bf)
        kh = np.empty((NHALO, 1, W), bf)
        for (rr, di, dj, q) in halo_rows:
            fhs[rr, :, :] = fpb[b, h0 + HS + q, :, dj:dj + W]
            kh[rr, 0, :] = kpb[b, 4 + h0 + HS + q - di, di * K + dj, :]
        fhs[50, :, :] = np.broadcast_to(biasb[:, None], (C, W))
        kh[50, 0, :] = bf(1.0)
        fhs2 = np.zeros((NH2, 16, W), bf)
        kh2 = np.zeros((NH2, 1, W), bf)
        for q4 in range(NQ):
            fhs2[0:NHALO, 4 * q4:4 * q4 + 4, :] = \
                fhs[:, CQ * q4:CQ * q4 + 4, :]
            fhs2[64:64 + NHALO, 4 * q4:4 * q4 + 4, :] = \
                fhs[:, CQ * q4 + 4:CQ * q4 + 8, :]
        kh2[0:NHALO] = kh
        kh2[64:64 + NHALO] = kh
        in_maps.append({
            "fa0": fa0, "fa1": fa1,
            "ks": np.ascontiguousarray(ks),
            "sst": sst2, "shal": shal2,
            "fhs": np.ascontiguousarray(fhs2),
            "kh": np.ascontiguousarray(kh2),
        })
    return in_maps


def _run(feat, kernel, bias, **run_kwargs):
    nc = _get_nc()
    in_maps = _prep_inputs(feat, kernel, bias)
    res = run_bass_kernel_spmd(nc, in_maps, core_ids=list(range(8)),
                               **run_kwargs)
    out = np.empty((B, H, C, W), np.float32)
    for core in range(8):
        b, hh = core // 2, core % 2
        out[b, hh * HS:(hh + 1) * HS] = res.results[core]["out"].astype(
            np.float32)
    return np.ascontiguousarray(out.transpose(0, 1, 3, 2)), res


def kernel(feat, kernel, bias):
    out, _ = _run(np.asarray(feat, np.float32), np.asarray(kernel, np.float32),
                  np.asarray(bias, np.float32))
    return out
